# revision 50
# baseline (speedup 1.0000x reference)
"""DFlashAttention Trainium2 kernel (8 NeuronCores, SPMD, no collectives).

Problem (hardcoded shapes): B=4, QL=1024, CL=3072, KL=4096, H=2048,
NH=16 q-heads, NKV=4 kv-heads, HD=128.

Sharding: core i = (batch b = i//2, head-group g = i%2). Each core computes
8 q-heads / 2 kv-heads for one batch and produces a partial o_proj output
(contraction over its head block of Wo); the host sums the two partials per
batch (the "all-reduce after o_proj", done on host).

Final design (lineage: v1 baseline 1104us -> 950 -> 774 -> 705 -> 625 ->
587us, ~1.88x):
  - Host pre-transposes kv/cos/sin to h-major (no PE transposes) and
    downcasts everything to bf16: ALL matmuls run bf16 (uniform PE mode, no
    fp32_mode=HIGH switches), fp32 accumulation in PSUM throughout
    (measured total error 6.8e-3 vs the 2e-2 gate).
  - rmsnorm: Square on ACT, partition-sum via ones-matmul,
    reciprocal_approx_fast, w folded into the rstd broadcast outer-product.
  - Attention: S^T pair into a 2-bank PSUM tile -> ONE [128,1024] exp
    (bf16 out); softmax denominators via DVE pair-sum tree (one selector
    matmul into a [16,512] PSUM bank per QUAD of k-tiles), ONE reciprocal
    at stage end; lag-2-pair software pipeline with alternating sT PSUM
    tags so PE never waits on a fresh exp; PV pairs share the V-tile
    stationary (LDWEIGHTS dedup is off in this toolchain, so stationary
    reuse between adjacent matmuls is what hides the reload).
  - V stays resident in SBUF (no HBM staging); startup DMAs split so the
    first projection matmuls start as soon as the first h-tiles land;
    output in bf16 (host upcasts and pair-sums).
"""
import os
import sys

sys.path.insert(0, "/opt/trn_rl_repo")

import numpy as np
import ml_dtypes

import concourse.bass as bass
import concourse.tile as tile
from concourse import bacc, mybir
from concourse.bass_utils import run_bass_kernel_spmd

# NOTE: walrus's redundant-LDWEIGHTS elimination (--enable-ldw-opt) is
# incompatible with bass-emitted InstLdweights, so every matmul reloads its
# stationary; adjacent same-stationary matmuls are the only dedup available.

f32 = mybir.dt.float32
bf16 = mybir.dt.bfloat16
AF = mybir.ActivationFunctionType

P = 128
H = 2048
HT = H // P          # 16 h-tiles
QL = 1024
CL = 3072
KL = CL + QL         # 4096
KT_N = KL // P       # 32 k-tiles
HD = 128
NHC = 8              # q heads per core
NKVC = 2             # kv heads per core
SCALE = HD ** -0.5
EPS = 1e-6

_NC = None


def build_nc():
    nc = bacc.Bacc("TRN2", target_bir_lowering=False, debug=False)

    kvT = nc.dram_tensor("kvt", [H, KL], bf16, kind="ExternalInput").ap()
    cosT = nc.dram_tensor("cost", [HD, KL], bf16, kind="ExternalInput").ap()
    sinT = nc.dram_tensor("sint", [HD, KL], bf16, kind="ExternalInput").ap()
    wq = nc.dram_tensor("wq", [H, NHC * HD], bf16, kind="ExternalInput").ap()
    wk = nc.dram_tensor("wk", [H, NKVC * HD], bf16, kind="ExternalInput").ap()
    wv = nc.dram_tensor("wv", [H, NKVC * HD], bf16, kind="ExternalInput").ap()
    wo = nc.dram_tensor("wo", [NHC * HD, H], bf16, kind="ExternalInput").ap()
    qnw = nc.dram_tensor("qnw", [1, HD], bf16, kind="ExternalInput").ap()
    knw = nc.dram_tensor("knw", [1, HD], bf16, kind="ExternalInput").ap()
    out = nc.dram_tensor("out", [QL, H], bf16, kind="ExternalOutput").ap()

    with tile.TileContext(nc) as tc:
        with tc.tile_pool(name="persist", bufs=1) as persist:
            # ---- constants ----
            rotm = persist.tile([P, P], bf16)
            ones_col = persist.tile([P, 1], bf16)
            ones_row = persist.tile([1, P], bf16)
            with tc.tile_pool(name="cscratch", bufs=1) as csp:
                rot_f = csp.tile([P, P], f32)
                nc.gpsimd.memset(rot_f, 0.0)
                # +1 where col = row + 64 (out[d'] = x[d'-64] for d' >= 64)
                nc.gpsimd.affine_select(
                    out=rot_f, in_=rot_f, compare_op=mybir.AluOpType.not_equal,
                    fill=1.0, base=64, pattern=[[-1, P]], channel_multiplier=1)
                # -1 where col = row - 64 (out[d'] = -x[d'+64] for d' < 64)
                nc.gpsimd.affine_select(
                    out=rot_f, in_=rot_f, compare_op=mybir.AluOpType.not_equal,
                    fill=-1.0, base=-64, pattern=[[-1, P]],
                    channel_multiplier=1)
                nc.vector.tensor_copy(rotm, rot_f)

                ones_f = csp.tile([P, P], f32)
                nc.vector.memset(ones_f, 1.0)
                nc.vector.tensor_copy(ones_col, ones_f[:, 0:1])
                nc.vector.tensor_copy(ones_row, ones_f[0:1, :])

            # dens selectors: sel_c[:, r, :] = [128,16] with col r = 1
            sel_c = persist.tile([P, 16, 16], bf16)
            nc.vector.memset(sel_c, 0.0)
            for r in range(16):
                nc.vector.memset(sel_c[:, r, r:r + 1], 1.0)
            # broadcast selectors: selB[:, r, :] = [16,128], row r = 1
            # (fill where partition == free_idx0; sign-symmetric so the
            # affine_select base-sign convention doesn't matter)
            selB = persist.tile([16, 16, P], bf16)
            with tc.tile_pool(name="selscr", bufs=1) as sscr:
                selB_f = sscr.tile([16, 16, P], f32)
                nc.gpsimd.memset(selB_f, 0.0)
                nc.gpsimd.affine_select(
                    out=selB_f, in_=selB_f,
                    compare_op=mybir.AluOpType.not_equal,
                    fill=1.0, base=0, pattern=[[-1, 16], [0, P]],
                    channel_multiplier=1)
                nc.vector.tensor_copy(selB, selB_f)

            qn_row = persist.tile([1, HD], bf16)
            nc.sync.dma_start(out=qn_row, in_=qnw)
            kn_row = persist.tile([1, HD], bf16)
            nc.sync.dma_start(out=kn_row, in_=knw)

            eps_sb = persist.tile([1, 1], f32)
            nc.vector.memset(eps_sb, EPS)

            # ---- persistent activations ----
            QT = persist.tile([P, NHC, QL], bf16)    # Q'^T  [d, head, q]
            KTt = persist.tile([P, NKVC, KL], bf16)  # K'^T  [d, kvh, k]
            V_sb = persist.tile([P, KT_N, NKVC * HD], bf16)  # [tok, kt, c]

            def norm_rope(ps, w_row, cosT_ap, sinT_ap, dst_ap, mid, psums):
                """ps [128,512] f32 PSUM -> dst_ap (bf16 SBUF): rmsnorm+rope.

                sq(ACT Square) -> ssq(PE ones-matmul) -> sqrt(ACT) ->
                recip_approx_fast(DVE) -> scl = w (x) rstd (PE outer) ->
                scl to SBUF (ACT) -> qn = ps*scl (DVE) -> rot (PE) ->
                t1,t2,add (DVE).
                """
                ssqp, sclp, rotp = psums
                sq = mid.tile([P, 512], bf16, tag="sq")
                nc.scalar.activation(sq, ps, func=AF.Square)
                ssq = ssqp.tile([1, 512], f32, tag="ssq")
                nc.tensor.matmul(ssq, ones_col, sq, start=True, stop=True)
                srt = mid.tile([1, 512], f32, tag="srt", bufs=1)
                nc.scalar.activation(srt, ssq, func=AF.Sqrt, scale=1.0 / HD,
                                     bias=eps_sb)
                rstd = mid.tile([1, 512], f32, tag="rstd", bufs=1)
                nc.vector.reciprocal_approx_fast(out=rstd, in_=srt)
                rstd_b = mid.tile([1, 512], bf16, tag="rstd_b", bufs=1)
                nc.vector.tensor_copy(rstd_b, rstd)
                scl_ps = sclp.tile([P, 512], f32, tag="scl_ps")
                nc.tensor.matmul(scl_ps, w_row, rstd_b, start=True, stop=True)
                scl = mid.tile([P, 512], f32, tag="scl", bufs=1)
                nc.scalar.activation(scl, scl_ps, func=AF.Copy)
                qn = mid.tile([P, 512], bf16, tag="qn")
                nc.vector.tensor_mul(qn, ps, scl)
                rot_ps = rotp.tile([P, 512], f32, tag="rot_ps")
                nc.tensor.matmul(rot_ps, rotm, qn, start=True, stop=True)
                t1 = mid.tile([P, 512], bf16, tag="t1", bufs=1)
                nc.vector.tensor_mul(t1, qn, cosT_ap)
                t2 = mid.tile([P, 512], bf16, tag="t2", bufs=1)
                nc.vector.tensor_mul(t2, rot_ps, sinT_ap)
                nc.vector.tensor_add(dst_ap, t1, t2)

            # ========= Stage QKV (Q folded into chunks 6-7) =========
            with tc.tile_pool(name="kv_str", bufs=3) as kvp, \
                 tc.tile_pool(name="kv_w", bufs=1) as wp, \
                 tc.tile_pool(name="q_w", bufs=2) as wqp, \
                 tc.tile_pool(name="kv_mid", bufs=2) as midp, \
                 tc.tile_pool(name="kv_cst", bufs=2) as cstp, \
                 tc.tile_pool(name="kv_proj", bufs=3, space="PSUM") as projp, \
                 tc.tile_pool(name="kv_pv", bufs=2, space="PSUM") as pvp, \
                 tc.tile_pool(name="kv_ssq", bufs=1, space="PSUM") as ssqp, \
                 tc.tile_pool(name="kv_scl", bufs=1, space="PSUM") as sclp, \
                 tc.tile_pool(name="kv_rot", bufs=1, space="PSUM") as rotp:
                # split + interleave the startup DMAs so the first projection
                # matmuls (which need only kvc h-tiles 0-3 and the wk first
                # half) can begin as early as possible
                kvT_r = kvT.rearrange("(ht p) k -> p ht k", p=P)
                wk_r = wk.rearrange("(ht p) c -> p ht c", p=P)
                wv_r = wv.rearrange("(ht p) c -> p ht c", p=P)
                kvc0 = kvp.tile([P, HT, 512], bf16, tag="kvc")
                wk_sb = wp.tile([P, HT, NKVC * HD], bf16)
                wv_sb = wp.tile([P, HT, NKVC * HD], bf16)
                nc.sync.dma_start(out=kvc0[:, 0:4], in_=kvT_r[:, 0:4, 0:512])
                nc.sync.dma_start(out=wk_sb[:, 0:8], in_=wk_r[:, 0:8])
                nc.sync.dma_start(out=kvc0[:, 4:8], in_=kvT_r[:, 4:8, 0:512])
                nc.sync.dma_start(out=wk_sb[:, 8:16], in_=wk_r[:, 8:16])
                nc.sync.dma_start(out=kvc0[:, 8:12],
                                  in_=kvT_r[:, 8:12, 0:512])
                nc.sync.dma_start(out=wv_sb[:, 0:8], in_=wv_r[:, 0:8])
                nc.sync.dma_start(out=kvc0[:, 12:16],
                                  in_=kvT_r[:, 12:16, 0:512])
                nc.sync.dma_start(out=wv_sb[:, 8:16], in_=wv_r[:, 8:16])
                for ch in range(8):
                    col = slice(ch * 512, (ch + 1) * 512)
                    if ch == 0:
                        kvc = kvc0
                    else:
                        kvc = kvp.tile([P, HT, 512], bf16, tag="kvc")
                        nc.sync.dma_start(out=kvc, in_=kvT_r[:, :, col])
                    cosT_c = cstp.tile([P, 512], bf16, tag="cosT")
                    nc.sync.dma_start(out=cosT_c, in_=cosT[:, col])
                    sinT_c = cstp.tile([P, 512], bf16, tag="sinT")
                    nc.sync.dma_start(out=sinT_c, in_=sinT[:, col])
                    # Q projection for the noise rows (chunks 6, 7) FIRST:
                    # the chain-free K/V projection matmuls issued after it
                    # then cover the trailing Q-norm dependency stalls
                    # instead of head-of-line blocking the ATT stage
                    if ch >= 6:
                        qc = ch - 6
                        for ct in range(NHC):
                            wq_t = wqp.tile([P, HT, P], bf16, tag="wq")
                            nc.sync.dma_start(
                                out=wq_t,
                                in_=wq[:, ct * P:(ct + 1) * P].rearrange(
                                    "(ht p) c -> p ht c", p=P))
                            psq = projp.tile([P, 512], f32, tag="proj")
                            for ht in range(HT):
                                nc.tensor.matmul(
                                    psq, wq_t[:, ht, :], kvc[:, ht, :],
                                    start=(ht == 0), stop=(ht == HT - 1))
                            norm_rope(psq, qn_row, cosT_c, sinT_c,
                                      QT[:, ct, qc * 512:(qc + 1) * 512],
                                      midp, (ssqp, sclp, rotp))
                    # K^T projection + norm + rope (per kv head = 128 rows)
                    for ckt in range(NKVC):
                        ps = projp.tile([P, 512], f32, tag="proj")
                        for ht in range(HT):
                            nc.tensor.matmul(
                                ps, wk_sb[:, ht, ckt * HD:(ckt + 1) * HD],
                                kvc[:, ht, :],
                                start=(ht == 0), stop=(ht == HT - 1))
                        norm_rope(ps, kn_row, cosT_c, sinT_c,
                                  KTt[:, ckt, col],
                                  midp, (ssqp, sclp, rotp))
                    # V projection (natural layout), resident in SBUF
                    for tt in range(4):
                        psv = pvp.tile([P, NKVC * HD], f32, tag="psv")
                        for ht in range(HT):
                            nc.tensor.matmul(
                                psv, kvc[:, ht, tt * P:(tt + 1) * P],
                                wv_sb[:, ht, :],
                                start=(ht == 0), stop=(ht == HT - 1))
                        nc.vector.tensor_copy(V_sb[:, ch * 4 + tt, :], psv)

            # ========= Stage ATT + O =========
            with tc.tile_pool(name="post", bufs=1) as postp:
                OTraw = postp.tile([P, NHC, QL], bf16)  # unnormalized O^T
                OT = postp.tile([P, NHC, QL], bf16)     # normalized O^T
                wo_sb = postp.tile([P, NHC, H], bf16)
                nc.sync.dma_start(
                    out=wo_sb,
                    in_=wo.rearrange("(ci p) n -> p ci n", p=P))
                rden = postp.tile([16, 512], f32)       # 1/dens  [(h,qc), q]
                rden_b = postp.tile([16, 512], bf16)
                _stage_att(nc, tc, OTraw, rden, rden_b, KTt, QT, V_sb,
                           sel_c)
                _stage_o(nc, tc, OTraw, OT, rden_b, selB, wo_sb, out)

    nc.compile()
    return nc


def _stage_att(nc, tc, OTraw, rden, rden_b, KTt, QT, V_sb, sel_b):
    with tc.tile_pool(name="at_et", bufs=8) as etp, \
         tc.tile_pool(name="at_ep", bufs=2) as epp, \
         tc.tile_pool(name="at_st", bufs=2, space="PSUM") as sTp, \
         tc.tile_pool(name="at_ot", bufs=1, space="PSUM") as oTp, \
         tc.tile_pool(name="at_den", bufs=1, space="PSUM") as denp:
        densP = denp.tile([16, 512], f32, tag="dens")
        ep_carry = [None]  # pair-sum awaiting its sibling (per quad)

        def pv_dens(peT_a, peT_b, poT, pr, ph):
            # PV for both kts of the pair (V-tile stationary shared per kt);
            # dens via DVE pair-sums folded once per QUAD of kts: the one or
            # two extra bf16 roundings put ~0.2% on den, negligible
            kvh = ph // 4
            for peT, pkt in ((peT_a, 2 * pr), (peT_b, 2 * pr + 1)):
                for qc in range(2):
                    sl = slice(qc * 512, (qc + 1) * 512)
                    nc.tensor.matmul(
                        poT[:, sl],
                        V_sb[:, pkt, kvh * HD:(kvh + 1) * HD],
                        peT[:, sl],
                        start=(pkt == 0), stop=(pkt == KT_N - 1))
            ep = epp.tile([P, QL], bf16, tag="ep")
            nc.vector.tensor_add(ep, peT_a, peT_b)
            if pr % 2 == 0:
                ep_carry[0] = ep
                return
            ep2 = epp.tile([P, QL], bf16, tag="ep2")
            nc.vector.tensor_add(ep2, ep_carry[0], ep)
            ep_carry[0] = None
            for qc in range(2):
                r = ph * 2 + qc
                sl = slice(qc * 512, (qc + 1) * 512)
                nc.tensor.matmul(
                    densP, sel_b[:, r, :], ep2[:, sl],
                    start=(ph == 0 and pr == 1 and qc == 0),
                    stop=(ph == NHC - 1 and pr == KT_N // 2 - 1 and qc == 1))

        for lh in range(NHC):
            kvh = lh // 4
            oT = oTp.tile([P, QL], f32, tag="oT")
            pend = []  # lag-2-pair pipeline: PE never waits on a fresh exp
            for pr in range(KT_N // 2):
                eTs = []
                for kt in (2 * pr, 2 * pr + 1):
                    # alternate explicit tags so the S(kt) write and the
                    # exp(kt-1) read never touch the same PSUM slot object
                    sT = sTp.tile([P, QL], f32, tag=f"sT{kt % 2}", bufs=1)
                    for qc in range(2):
                        nc.tensor.matmul(
                            sT[:, qc * 512:(qc + 1) * 512],
                            KTt[:, kvh, kt * P:(kt + 1) * P],
                            QT[:, lh, qc * 512:(qc + 1) * 512],
                            start=True, stop=True)
                    eT = etp.tile([P, QL], bf16, tag="eT")
                    nc.scalar.activation(eT, sT, func=AF.Exp, scale=SCALE)
                    eTs.append(eT)
                pend.append((eTs[0], eTs[1], oT, pr, lh))
                if len(pend) > 3:
                    pv_dens(*pend.pop(0))
            for args in pend:
                pv_dens(*args)
            # copy-out on ACT: it starts right after the last PV instead of
            # queueing behind DVE's ep-add backlog, so the next head's first
            # PV (which reuses the single oT buffer) unblocks sooner
            for qc in range(2):
                sl = slice(qc * 512, (qc + 1) * 512)
                nc.scalar.activation(OTraw[:, lh, sl], oT[:, sl],
                                     func=AF.Copy)
        # all dens accumulated: one fast reciprocal for all 16 (h,qc) rows
        nc.vector.reciprocal_approx_fast(out=rden, in_=densP)
        nc.vector.tensor_copy(rden_b, rden)


def _stage_o(nc, tc, OTraw, OT, rden_b, selB, wo_sb, out):
    with tc.tile_pool(name="o_out", bufs=4) as outp, \
         tc.tile_pool(name="o_bc", bufs=2, space="PSUM") as bcp, \
         tc.tile_pool(name="o_ps0", bufs=2, space="PSUM") as opsA, \
         tc.tile_pool(name="o_ps1", bufs=2, space="PSUM") as opsB:
        # normalize: OT = OTraw * broadcast(rden[h*2+qc])
        for qc in range(2):
            for h in range(NHC):
                r = h * 2 + qc
                sl = slice(qc * 512, (qc + 1) * 512)
                bc = bcp.tile([P, 512], f32, tag="bc")
                nc.tensor.matmul(bc, selB[:, r, :], rden_b,
                                 start=True, stop=True)
                nc.vector.tensor_mul(OT[:, h, sl], OTraw[:, h, sl], bc)
        # o_proj: out[q, n] = sum_ci OT[:, ci, q].T @ wo[:, ci, n]
        # (qt-outer: qt 0-3 depend only on the qc0 normalize muls, so the
        # first o_proj matmuls start after 8 muls instead of 16)
        for qt in range(8):
            for half in range(2):
                ps0 = opsA.tile([P, 512], f32, tag="ops0")
                ps1 = opsB.tile([P, 512], f32, tag="ops1")
                pss = (ps0, ps1)
                for ci in range(NHC):
                    for nch in range(2):
                        nc.tensor.matmul(
                            pss[nch], OT[:, ci, qt * P:(qt + 1) * P],
                            wo_sb[:, ci,
                                  half * 1024 + nch * 512:
                                  half * 1024 + (nch + 1) * 512],
                            start=(ci == 0), stop=(ci == NHC - 1))
                ob = outp.tile([P, 1024], bf16, tag="ob")
                nc.scalar.activation(ob[:, 0:512], ps0, func=AF.Copy)
                nc.vector.tensor_copy(ob[:, 512:1024], ps1)
                nc.sync.dma_start(
                    out=out[qt * P:(qt + 1) * P,
                            half * 1024:(half + 1) * 1024],
                    in_=ob)


def _get_nc():
    global _NC
    if _NC is None:
        _NC = build_nc()
    return _NC


def _make_in_maps(noise, ctx, cos, sin, Wq, Wk, Wv, Wo, qn_w, kn_w):
    bf = ml_dtypes.bfloat16
    noise = np.asarray(noise, np.float32)
    ctx = np.asarray(ctx, np.float32)
    cos = np.asarray(cos, np.float32)
    sin = np.asarray(sin, np.float32)
    Wq = np.asarray(Wq, np.float32).astype(bf)
    Wk = np.asarray(Wk, np.float32).astype(bf)
    Wv = np.asarray(Wv, np.float32).astype(bf)
    Wo = np.asarray(Wo, np.float32).astype(bf)
    qn_w = np.asarray(qn_w, np.float32).reshape(1, HD).astype(bf)
    kn_w = np.asarray(kn_w, np.float32).reshape(1, HD).astype(bf)
    B = noise.shape[0]
    in_maps = []
    for b in range(B):
        kvT_b = np.ascontiguousarray(
            np.concatenate([ctx[b], noise[b]], axis=0).T).astype(bf)
        cosT_b = np.ascontiguousarray(cos[b].T).astype(bf)
        sinT_b = np.ascontiguousarray(sin[b].T).astype(bf)
        for g in range(2):
            in_maps.append({
                "kvt": kvT_b,
                "cost": cosT_b,
                "sint": sinT_b,
                "wq": np.ascontiguousarray(Wq[:, g * 1024:(g + 1) * 1024]),
                "wk": np.ascontiguousarray(Wk[:, g * 256:(g + 1) * 256]),
                "wv": np.ascontiguousarray(Wv[:, g * 256:(g + 1) * 256]),
                "wo": np.ascontiguousarray(Wo[g * 1024:(g + 1) * 1024, :]),
                "qnw": qn_w,
                "knw": kn_w,
            })
    return in_maps


def _install_profile_hook():
    """Provide antenv.axon_hooks (absent in this container) so
    run_bass_kernel_spmd(trace=True) can NTFF-profile via libaxon_pjrt."""
    import types
    if "antenv.axon_hooks" not in sys.modules:
        import antenv
        mod = types.ModuleType("antenv.axon_hooks")
        _state = {}
        mod.set_axon_ntff_profile_hook = lambda h: _state.__setitem__("h", h)
        mod.get_axon_ntff_profile_hook = lambda: _state.get("h")
        sys.modules["antenv.axon_hooks"] = mod
        antenv.axon_hooks = mod
        from trn_agent_boot.trn_boot import _ntff_profile_via_ctypes
        mod.set_axon_ntff_profile_hook(
            _ntff_profile_via_ctypes("/opt/axon/libaxon_pjrt.so"))
    import concourse.bass_utils as bu
    bu.upload_artifacts = lambda tmpdir: tmpdir


def run(inputs, trace=False, tmpdir=None):
    """Run on 8 cores; returns (output [4,1024,2048], exec_time_ns or None)."""
    nc = _get_nc()
    in_maps = _make_in_maps(**inputs)
    if trace:
        _install_profile_hook()
    res = run_bass_kernel_spmd(nc, in_maps, core_ids=list(range(8)),
                               trace=trace, tmpdir=tmpdir,
                               trace_cores=[0] if trace else None)
    outs = [np.asarray(res.results[i]["out"], dtype=np.float32)
            for i in range(8)]
    full = np.stack([outs[2 * b] + outs[2 * b + 1] for b in range(4)], axis=0)
    return full.astype(np.float32), res


def kernel(**inputs):
    out, _ = run(inputs, trace=False)
    return out


def summarize_trace(res, top=30):
    """Per-engine busy time + top source lines by total duration."""
    if not res.instructions_and_trace:
        print("no trace")
        return
    insts, trace_path = res.instructions_and_trace
    from collections import defaultdict
    eng_busy = defaultdict(int)
    eng_n = defaultdict(int)
    line_cost = defaultdict(int)
    t0 = min(i.timestamp for i in insts)
    t1 = max(i.end_timestamp for i in insts)
    for i in insts:
        e = str(i.engine)
        eng_busy[e] += int(i.duration)
        eng_n[e] += 1
        line_cost[(e, str(i.op_name), str(i.source_line))] += int(i.duration)
    span = t1 - t0
    print(f"trace: {trace_path}")
    print(f"span: {span} ns")
    for e in sorted(eng_busy, key=lambda e: -eng_busy[e]):
        print(f"  {e:12s} busy {eng_busy[e]:>10} ns "
              f"({100.0 * eng_busy[e] / span:5.1f}%)  n={eng_n[e]}")
    print("top cost lines:")
    for (e, op, line), c in sorted(line_cost.items(),
                                   key=lambda kv: -kv[1])[:top]:
        print(f"  {c:>10} ns  {e:10s} {op:22s} {line}")
